# revision 2
# baseline (speedup 1.0000x reference)
"""DiceLoss kernel v2 for 8 Trainium2 NeuronCores — fp8 + PE-centric.

Reference computation:
    inter[b,c] = sum_p pred[b,c,p] * target[b,c,p]          # [4, 8]
    denom      = sum(pred) + sum(target) + 1.0              # scalar
    loss_bc    = 2 * (inter + 1) / denom
    total      = sum_b( sum_c(loss_bc[b]) * 8**(b-4) ) / 4
    out        = 1 - total

Numerics: inputs are uniform[0,1); host-casting them to fp8-e4m3 gives
rel err ~2e-8 on the final loss (quantization noise averages out over
16.8M samples; the 1-total leverage shrinks it further) — far inside
the 2e-2 gate while quartering HBM traffic, the binding constraint.

Sharding: flatten (b,c) -> 32 rows of 2M pixels; core k takes rows
4k..4k+3 ("groups" 0..3).  Per group the 2M pixels are packed into 130
PE tiles of [K=128 partitions, 128 cols] where col 127 of every tile
is 1.0 (and the data tail is zero-padded).  For each tile the PE runs
matmul(psum_g, lhsT=pred_tile, rhs=targ_tile) accumulating all 130
tiles of a group into one [128,128] PSUM region:
    psum_g[i,j]   = sum_t sum_k pred[k,t,i] * targ[k,t,j]
    diag(psum_g)  -> per-column dot products   (sum -> inter[g])
    psum_g[:,127] -> per-column pred sums      (sum -> sum(pred))
    psum_g[127,:] -> per-column targ sums      (sum -> sum(targ))
so the PE computes the dots AND both global sums in one fp8 matmul
stream (~35us/core), under the fp8 DMA floor (~47us/core).  DVE only
extracts: diag via a masked reduce against an identity matrix, plus a
column copy and a row reduce per group (<1us total).  Host folds the
[128,12] per-core result into the scalar loss.
"""

from contextlib import ExitStack

import numpy as np
import ml_dtypes

N, C, P = 4, 8, 2097152
NCORES = 8
ROWS = N * C                      # 32 (b,c) rows
RPC = ROWS // NCORES              # 4 rows (groups) per core
COLS = P // 128                   # 16384 data cols per group
NT = 130                          # PE tiles per group (130*127 >= 16384)
TPG = NT * 128                    # 16640 slab cols per group
SLAB_W = RPC * TPG                # 66560 slab cols per core
GUARD = 16                        # unread guard rows flanking dram slabs
SUB = 5                           # DMA sub-pieces per group per tensor
SUBT = NT // SUB                  # 26 tiles per sub-piece
SUBW = SUBT * 128                 # 3328 cols per sub-piece

F8 = ml_dtypes.float8_e4m3

_CACHE = {}


def _build_bass():
    import concourse.bass as bass
    import concourse.mybir as mybir

    f32 = mybir.dt.float32
    f8 = mybir.dt.float8e4
    nc = bass.Bass("TRN2", target_bir_lowering=False, debug=False,
                   num_devices=NCORES)

    pred = nc.dram_tensor("pred", [128 + 2 * GUARD, SLAB_W], f8,
                          kind="ExternalInput").ap()
    targ = nc.dram_tensor("target", [128 + 2 * GUARD, SLAB_W], f8,
                          kind="ExternalInput").ap()
    ident = nc.dram_tensor("ident", [128, 128], f32,
                           kind="ExternalInput").ap()
    out = nc.dram_tensor("out", [128, 12], f32, kind="ExternalOutput").ap()

    predf = pred[GUARD:GUARD + 128, :]
    targf = targ[GUARD:GUARD + 128, :]

    AX = mybir.AxisListType.X
    MUL = mybir.AluOpType.mult

    with ExitStack() as ctx:
        e = ctx.enter_context
        pred_sl = [e(nc.sbuf_tensor(f"pred_sl{i}", [128, TPG], f8))
                   for i in range(2)]
        targ_sl = [e(nc.sbuf_tensor(f"targ_sl{i}", [128, TPG], f8))
                   for i in range(2)]
        ident_sb = e(nc.sbuf_tensor([128, 128], f32))
        finals = e(nc.sbuf_tensor([128, 12], f32))
        dummy = e(nc.sbuf_tensor([128, 1], f32))
        ps = [e(nc.psum_tensor(f"ps{g}", [128, 128], f32))
              for g in range(RPC)]

        # one sem per (slot, sub-piece): every wait threshold equals the
        # total inc count of ALL DMAs ever issued on that sem, so a lagging
        # SDMA engine cannot be masked by faster engines racing ahead
        # (15 engines x all chunks < 16 x chunks_needed).
        ss = [[e(nc.semaphore(f"ss{p}_{s}")) for s in range(SUB)]
              for p in range(2)]
        s_id = e(nc.semaphore())   # ident loaded
        s_pe = e(nc.semaphore())   # PE groups done
        s_dve = e(nc.semaphore())  # DVE groups extracted
        s_out = e(nc.semaphore())  # output stored

        block = e(nc.Block(no_gpsimd_drain=True))

        @block.sync
        def _(sync):
            for g in range(RPC):
                p = g % 2
                if g >= 2:
                    sync.wait_ge(s_pe, g - 1)   # slot's previous group done
                base = g * TPG
                for s in range(SUB):
                    o = s * SUBW
                    sync.dma_start(
                        pred_sl[p][:, o:o + SUBW],
                        predf[:, base + o:base + o + SUBW],
                    ).then_inc(ss[p][s], 16)
                    sync.dma_start(
                        targ_sl[p][:, o:o + SUBW],
                        targf[:, base + o:base + o + SUBW],
                    ).then_inc(ss[p][s], 16)
            sync.wait_ge(s_dve, RPC)
            sync.dma_start(out, finals[:]).then_inc(s_out, 16)

        @block.tensor
        def _(tensor):
            for g in range(RPC):
                p = g % 2
                gen = g // 2
                for s in range(SUB):
                    tensor.wait_ge(ss[p][s], 32 * (gen + 1))
                    for t in range(SUBT):
                        ti = s * SUBT + t
                        mm = nc.tensor.matmul(
                            ps[g][:],
                            pred_sl[p][:, ti * 128:(ti + 1) * 128],
                            targ_sl[p][:, ti * 128:(ti + 1) * 128],
                            start=(ti == 0),
                            stop=(ti == NT - 1),
                        )
                        if ti == NT - 1:
                            mm.then_inc(s_pe, 1)

        @block.vector
        def _(vector):
            nc.vector.memset(finals[:], 0.0)
            vector.wait_ge(s_id, 16)
            for g in range(RPC):
                vector.wait_ge(s_pe, g + 1)
                nc.vector.scalar_tensor_tensor(
                    out=dummy[:, 0:1].broadcast_to((128, 128)),
                    in0=ps[g][:],
                    scalar=1.0,
                    in1=ident_sb[:],
                    op0=MUL,
                    op1=MUL,
                    accum_out=finals[:, g:g + 1],
                )
                nc.vector.tensor_copy(finals[:, 4 + g:5 + g],
                                      ps[g][:, 127:128])
                nc.vector.reduce_sum(finals[:, 8 + g:9 + g],
                                     ps[g][:, 0:127],
                                     axis=AX).then_inc(s_dve, 1)

        @block.scalar
        def _(scalar):
            scalar.dma_start(ident_sb[:], ident).then_inc(s_id, 16)

    return nc


def _pack(core_rows: np.ndarray) -> np.ndarray:
    """[RPC, P] fp8 rows -> guarded [128+2G, SLAB_W] fp8 slab."""
    slab = np.zeros((128 + 2 * GUARD, SLAB_W), dtype=F8)
    body = slab[GUARD:GUARD + 128]
    one = np.array(1.0, dtype=F8)
    packed = np.zeros((128, NT, 128), dtype=F8)
    pad = np.zeros((128, NT * 127), dtype=F8)
    for g in range(RPC):
        pad[:, :COLS] = core_rows[g].reshape(COLS, 128).T
        packed[:, :, :127] = pad.reshape(128, NT, 127)
        packed[:, :, 127] = one
        body[:, g * TPG:(g + 1) * TPG] = packed.reshape(128, TPG)
    return slab


def _make_in_maps(pred: np.ndarray, target: np.ndarray):
    predr = np.asarray(pred, dtype=np.float32).reshape(ROWS, P).astype(F8)
    targr = np.asarray(target, dtype=np.float32).reshape(ROWS, P).astype(F8)
    ident = np.eye(128, dtype=np.float32)
    maps = []
    for k in range(NCORES):
        maps.append({
            "pred": _pack(predr[k * RPC:(k + 1) * RPC]),
            "target": _pack(targr[k * RPC:(k + 1) * RPC]),
            "ident": ident,
        })
    return maps


def _run(pred: np.ndarray, target: np.ndarray, trace: bool = False):
    from concourse.bass_utils import run_bass_kernel_spmd

    if "nc" not in _CACHE:
        _CACHE["nc"] = _build_bass()
    nc = _CACHE["nc"]
    in_maps = _make_in_maps(pred, target)
    return run_bass_kernel_spmd(nc, in_maps, core_ids=list(range(NCORES)),
                                trace=trace)


def _combine(results) -> np.ndarray:
    inter = np.empty(ROWS, dtype=np.float64)
    sums = 0.0
    for k in range(NCORES):
        o = np.asarray(results[k]["out"], dtype=np.float64)   # [128, 12]
        for g in range(RPC):
            inter[k * RPC + g] = o[0:127, g].sum()
            sums += o[0:127, 4 + g].sum() + o[127, 8 + g]
    denom = sums + 1.0
    loss_bc = 2.0 * (inter.reshape(N, C) + 1.0) / denom
    weights = np.float64(C) ** (np.arange(N, dtype=np.float64) - N)
    total = (loss_bc.sum(axis=1) * weights).sum() / N
    return np.array(1.0 - total, dtype=np.float32)


def kernel(pred: np.ndarray, target: np.ndarray) -> np.ndarray:
    res = _run(pred, target, trace=False)
    return _combine(res.results)
